# revision 27
# baseline (speedup 1.0000x reference)
"""Trainium2 Bass kernel for AdaptiveAttentionTransformerBlock (sparse attention).

Self-contained: hardcodes shapes/sharding. Sequence-sharded across 8 cores
(2 batches x 4 sequence slices of 512 tokens); no collectives needed because
the attention mask (block-local 256 | sliding window 128 | 4 global tokens,
causal) only requires a 128-token halo plus the 4 global tokens per slice.

Per-core pipeline (all matmuls bf16 inputs -> f32 PSUM accum):
  xT [1024, 644] (halo 128 | own 512 | global 4) feature-major
  Q/K proj -> feature-major [feat, tok]; V proj -> token-major [tok, feat]
  RoPE via const matmul R (rotate_half) + elementwise cos/sin (f32)
  scoresT [k, q] computed directly (no attn transpose needed for AV)
  exp without max-subtraction (logits are O(5) for this data), masks are
  0/1 multiplies after exp, softmax denominator via an appended ones-column
  in V (row 64 of the AV output), normalization by reciprocal broadcast
  Out proj from feature-major attn output, y [512, 1024] f32 per core.
"""
import sys

sys.path.insert(0, "/opt/trn_rl_repo")

import numpy as np
import ml_dtypes

import concourse.bacc as bacc
import concourse.bass as bass
import concourse.mybir as mybir
import concourse.tile as tile
from concourse import bass_utils

BF16 = ml_dtypes.bfloat16
F32 = mybir.dt.float32
BF = mybir.dt.bfloat16

EMB, HEADS, HD = 1024, 16, 64
B, S = 2, 2048
SCALE = HD ** -0.5
CTX = 644  # 128 halo + 512 own + 4 global
MUL = mybir.AluOpType.mult
ADD = mybir.AluOpType.add
EXP = mybir.ActivationFunctionType.Exp


def _build_graph(dbg=False):
    nc = bacc.Bacc("TRN2", target_bir_lowering=False, debug=False)

    D = {}
    D["xt"] = nc.dram_tensor("xt", [EMB, CTX], BF, kind="ExternalInput")
    for w in ("wq", "wk", "wv", "wo"):
        D[w] = nc.dram_tensor(w, [EMB, EMB], BF, kind="ExternalInput")
    D["cosq"] = nc.dram_tensor("cosq", [128, 512], BF, kind="ExternalInput")
    D["sinq"] = nc.dram_tensor("sinq", [128, 512], BF, kind="ExternalInput")
    D["cosk"] = nc.dram_tensor("cosk", [128, CTX], BF, kind="ExternalInput")
    D["sink"] = nc.dram_tensor("sink", [128, CTX], BF, kind="ExternalInput")
    D["ident"] = nc.dram_tensor("ident", [128, 128], BF, kind="ExternalInput")
    D["rmat"] = nc.dram_tensor("rmat", [128, 128], BF, kind="ExternalInput")
    D["mtri"] = nc.dram_tensor("mtri", [128, 128], BF, kind="ExternalInput")
    D["mwin"] = nc.dram_tensor("mwin", [128, 128], BF, kind="ExternalInput")
    D["m0"] = nc.dram_tensor("m0", [128, 128], BF, kind="ExternalInput")
    D["gmask"] = nc.dram_tensor("gmask", [128, 256], BF, kind="ExternalInput")
    D["out"] = nc.dram_tensor("out", [512, EMB], F32, kind="ExternalOutput")
    if dbg:
        D["dbg_qrot0"] = nc.dram_tensor("dbg_qrot0", [128, 512], BF, kind="ExternalOutput")
        D["dbg_krot0"] = nc.dram_tensor("dbg_krot0", [128, CTX], BF, kind="ExternalOutput")
        D["dbg_vsb1"] = nc.dram_tensor("dbg_vsb1", [128, 16, 65], BF, kind="ExternalOutput")
        D["dbg_vsb5"] = nc.dram_tensor("dbg_vsb5", [128, 16, 65], BF, kind="ExternalOutput")
        D["dbg_attg0"] = nc.dram_tensor("dbg_attg0", [128, 512], BF, kind="ExternalOutput")
        D["dbg_aT0"] = nc.dram_tensor("dbg_aT0", [128, 512], BF, kind="ExternalOutput")

    with tile.TileContext(nc) as tc:
        _body(nc, tc, D, dbg=dbg)

    nc.compile()
    return nc


def _body(nc, tc, D, dbg=False):
    from contextlib import ExitStack
    es = ExitStack()
    cp = es.enter_context(tc.tile_pool(name="const", bufs=1))
    pjp = tc.tile_pool(name="proj", bufs=5, space=bass.MemorySpace.PSUM)
    wp = pjp.__enter__()
    sp = es.enter_context(tc.tile_pool(name="sb", bufs=4))
    atp = es.enter_context(tc.tile_pool(name="att", bufs=6))

    # ---- persistent SBUF tiles ----
    xt = [cp.tile([128, CTX], BF, tag=f"xt{i}", name=f"xt{i}") for i in range(8)]
    wq = [cp.tile([128, EMB], BF, tag=f"wq{i}", name=f"wq{i}") for i in range(8)]
    wk = [cp.tile([128, EMB], BF, tag=f"wk{i}", name=f"wk{i}") for i in range(8)]
    wv = [cp.tile([128, EMB], BF, tag=f"wv{i}", name=f"wv{i}") for i in range(8)]
    wo = [cp.tile([128, EMB], BF, tag=f"wo{i}", name=f"wo{i}") for i in range(8)]
    cosq = cp.tile([128, 512], BF, tag="cosq")
    sinq = cp.tile([128, 512], BF, tag="sinq")
    cosk = cp.tile([128, CTX], BF, tag="cosk")
    sink = cp.tile([128, CTX], BF, tag="sink")
    ident = cp.tile([128, 128], BF, tag="ident")
    rmat = cp.tile([128, 128], BF, tag="rmat")
    mtri = cp.tile([128, 128], BF, tag="mtri")
    mwin = cp.tile([128, 128], BF, tag="mwin")
    m0 = cp.tile([128, 128], BF, tag="m0")
    gmask = cp.tile([128, 256], BF, tag="gmask")
    zbias = cp.tile([128, 1], F32, tag="zbias")
    qrot = [cp.tile([128, 512], BF, tag=f"qrot{i}", name=f"qrot{i}") for i in range(8)]
    krot = [cp.tile([128, CTX], BF, tag=f"krot{i}", name=f"krot{i}") for i in range(8)]
    vsb = [cp.tile([128, 16, 65], BF, tag=f"vsb{i}", name=f"vsb{i}") for i in range(6)]
    aT = [cp.tile([128, 512], BF, tag=f"aT{i}", name=f"aT{i}") for i in range(8)]
    ysb = [cp.tile([128, EMB], F32, tag=f"ysb{i}", name=f"ysb{i}") for i in range(4)]

    # ---- DMA loads (spread across engines for parallel queues) ----
    engs = [nc.sync, nc.scalar]
    di = 0

    def dma(dst, src):
        nonlocal di
        engs[di % len(engs)].dma_start(out=dst, in_=src)
        di += 1

    for i in range(8):
        dma(xt[i][:], D["xt"][128 * i:128 * (i + 1), :])
    for tiles, name in ((wv, "wv"), (wq, "wq"), (wk, "wk")):
        for i in range(8):
            dma(tiles[i][:], D[name][128 * i:128 * (i + 1), :])
    for t, name in ((cosq, "cosq"), (sinq, "sinq"), (cosk, "cosk"), (sink, "sink"),
                    (rmat, "rmat"), (mtri, "mtri"), (mwin, "mwin"), (m0, "m0"),
                    (gmask, "gmask"), (ident, "ident")):
        dma(t[:], D[name][:])
    for i in range(8):
        dma(wo[i][:], D["wo"][128 * i:128 * (i + 1), :])

    nc.vector.memset(zbias[:], 0.0)
    for t in range(6):
        nc.vector.memset(vsb[t][:, :, 64:65], 1.0)

    # ---- V projection (token-major [tok, vfeat]) ----
    for t in range(6):
        tok0, tw = (t * 128, 128) if t < 5 else (640, 4)
        for half in range(2):
            vp = wp.tile([128, 512], F32, tag="work", name="vp")
            for e in range(8):
                nc.tensor.matmul(vp[0:tw, :], lhsT=xt[e][:, tok0:tok0 + tw],
                                 rhs=wv[e][:, half * 512:(half + 1) * 512],
                                 start=(e == 0), stop=(e == 7))
            nc.vector.tensor_copy(
                vsb[t][0:tw, half * 8:(half + 1) * 8, 0:64],
                vp[0:tw, :].rearrange("p (h d) -> p h d", h=8))
    # glob AV matmuls for odd heads read rhs at base partition 64, and matmul
    # requires lhsT/rhs bases to match: replicate glob v rows to partition 64
    nc.sync.dma_start(out=vsb[5][64:68, :, :], in_=vsb[5][0:4, :, :])

    # ---- Q/K projections + RoPE (feature-major [feat, tok]) ----
    def proj_rope(hp, wtiles, col0, cw, ctile, stile, rot_out):
        """project feature chunk hp over token cols [col0, col0+cw), rope it"""
        pp = wp.tile([128, 512], F32, tag="work", name="pp")
        for e in range(8):
            nc.tensor.matmul(pp[:, 0:cw], lhsT=wtiles[e][:, hp * 128:(hp + 1) * 128],
                             rhs=xt[e][:, col0:col0 + cw],
                             start=(e == 0), stop=(e == 7))
        psb = sp.tile([128, 512], BF, tag="qsb", name="psb")
        nc.vector.tensor_copy(psb[:, 0:cw], pp[:, 0:cw])
        rq = wp.tile([128, 512], F32, tag="work", name="rq")
        nc.tensor.matmul(rq[:, 0:cw], lhsT=rmat[:], rhs=psb[:, 0:cw],
                         start=True, stop=True)
        rqb = sp.tile([128, 512], BF, tag="rqb", name="rqb")
        nc.vector.tensor_copy(rqb[:, 0:cw], rq[:, 0:cw])
        t0 = sp.tile([128, 512], BF, tag="t0", name="t0")
        nc.vector.tensor_tensor(out=t0[:, 0:cw], in0=psb[:, 0:cw],
                                in1=ctile[:, col0:col0 + cw], op=MUL)
        t1 = sp.tile([128, 512], BF, tag="t1", name="t1")
        nc.vector.tensor_tensor(out=t1[:, 0:cw], in0=rqb[:, 0:cw],
                                in1=stile[:, col0:col0 + cw], op=MUL)
        nc.vector.tensor_tensor(out=rot_out, in0=t0[:, 0:cw], in1=t1[:, 0:cw], op=ADD)

    for hp in range(8):
        # q: own tokens are xt cols 128:640; cosq indexed 0:512; scale folded
        pp = wp.tile([128, 512], F32, tag="work", name="pp")
        for e in range(8):
            nc.tensor.matmul(pp[:], lhsT=wq[e][:, hp * 128:(hp + 1) * 128],
                             rhs=xt[e][:, 128:640], start=(e == 0), stop=(e == 7))
        psb = sp.tile([128, 512], BF, tag="qsb", name="psb")
        nc.vector.tensor_copy(psb[:], pp[:])
        rq = wp.tile([128, 512], F32, tag="work", name="rq")
        nc.tensor.matmul(rq[:], lhsT=rmat[:], rhs=psb[:], start=True, stop=True)
        rqb = sp.tile([128, 512], BF, tag="rqb", name="rqb")
        nc.vector.tensor_copy(rqb[:], rq[:])
        t0 = sp.tile([128, 512], BF, tag="t0", name="t0")
        nc.vector.tensor_tensor(out=t0[:], in0=psb[:], in1=cosq[:], op=MUL)
        t1 = sp.tile([128, 512], BF, tag="t1", name="t1")
        nc.vector.tensor_tensor(out=t1[:], in0=rqb[:], in1=sinq[:], op=MUL)
        nc.vector.tensor_tensor(out=qrot[hp][:], in0=t0[:], in1=t1[:], op=ADD)
        # k: full context in two col chunks
        for (c0, cw) in ((0, 512), (512, 132)):
            proj_rope(hp, wk, c0, cw, cosk, sink, krot[hp][:, c0:c0 + cw])


    # ---- attention ----
    # glob scoresT batched per 4-head group: [16, 512] = 4 heads x 4 glob-k
    # rows, all 512 q columns in one matmul per head; exp'd + core-masked once.
    # two heads per tile: head parity p -> glob rows at partitions 64p..64p+3
    # (PE matmul out base partition must be 0/32/64)
    attg = [cp.tile([128, 512], BF, tag=f"attg{g}", name=f"attg{g}") for g in range(8)]
    for hp in range(8):
        gp = wp.tile([128, 512], F32, tag="work", name="gp")
        for p in range(2):
            dsl = slice(p * 64, p * 64 + 64)
            nc.tensor.matmul(gp[64 * p:64 * p + 4, :], lhsT=krot[hp][dsl, 640:644],
                             rhs=qrot[hp][dsl, :], start=True, stop=True)
        nc.scalar.activation(attg[hp][:], gp[:], EXP, bias=zbias[:])
        # global cols duplicate tile-0 keys for slices 0/1 when si == 0
        nc.gpsimd.tensor_tensor(out=attg[hp][:, 0:256], in0=attg[hp][:, 0:256],
                                in1=gmask[:], op=MUL)

    pjp.__exit__(None, None, None)
    wp = es.enter_context(tc.tile_pool(name="work", bufs=3, space=bass.MemorySpace.PSUM))
    avp = es.enter_context(tc.tile_pool(name="avp", bufs=3, space=bass.MemorySpace.PSUM))
    tpp = es.enter_context(tc.tile_pool(name="tpp", bufs=2, space=bass.MemorySpace.PSUM))

    for Ic in range(4):
        for g in range(4):
            for j in range(4):
                h = 4 * g + j
                hp, po = h // 2, (h % 2) * 64
                dsl = slice(po, po + 64)
                q_ap = qrot[hp][dsl, Ic * 128:(Ic + 1) * 128]       # [64, 128]
                st = wp.tile([128, 256], F32, tag="work", name="st")
                nc.tensor.matmul(st[:, 0:128],
                                 lhsT=krot[hp][dsl, Ic * 128:Ic * 128 + 128],
                                 rhs=q_ap, start=True, stop=True)
                nc.tensor.matmul(st[:, 128:256],
                                 lhsT=krot[hp][dsl, 128 + Ic * 128:256 + Ic * 128],
                                 rhs=q_ap, start=True, stop=True)
                att = atp.tile([128, 256], BF, tag="att", name="att")
                nc.scalar.activation(att[:], st[:], EXP, bias=zbias[:])
                nc.gpsimd.tensor_tensor(out=att[:, 128:256], in0=att[:, 128:256],
                                        in1=mtri[:], op=MUL)
                if Ic == 0:
                    nc.gpsimd.tensor_tensor(out=att[:, 0:128], in0=att[:, 0:128],
                                            in1=m0[:], op=MUL)
                elif Ic == 2:
                    nc.gpsimd.tensor_tensor(out=att[:, 0:128], in0=att[:, 0:128],
                                            in1=mwin[:], op=MUL)
                # q-major AV: out [128q, 65]; att chunks are the stationaries,
                # denominator lands as per-partition column 64
                av = avp.tile([128, 65], F32, tag="av", name="av")
                nc.tensor.matmul(av[:], lhsT=att[:, 0:128], rhs=vsb[Ic][:, h, 0:65],
                                 start=True, stop=False)
                nc.tensor.matmul(av[:], lhsT=att[:, 128:256],
                                 rhs=vsb[Ic + 1][:, h, 0:65], start=False, stop=False)
                nc.tensor.matmul(av[:], lhsT=attg[hp][po:po + 4, Ic * 128:(Ic + 1) * 128],
                                 rhs=vsb[5][po:po + 4, h, 0:65],
                                 start=False, stop=True)
                rec = sp.tile([128, 1], F32, tag="rec", name="rec")
                nc.vector.reciprocal(out=rec[:], in_=av[:, 64:65])
                nq = sp.tile([128, 64], BF, tag="nq", name="nq")
                nc.vector.tensor_scalar(out=nq[:], in0=av[:, 0:64], scalar1=rec[:],
                                        scalar2=None, op0=MUL)
                tp = tpp.tile([64, 128], BF, tag="tp", name="tp")
                nc.tensor.transpose(tp[:], nq[:], ident[:])
                nc.vector.tensor_copy(aT[hp][dsl, Ic * 128:(Ic + 1) * 128], tp[:])
        # ---- out projection for this q-tile ----
        for half in range(2):
            yp = wp.tile([128, 512], F32, tag="work", name="yp")
            for fc in range(8):
                nc.tensor.matmul(yp[:], lhsT=aT[fc][:, Ic * 128:(Ic + 1) * 128],
                                 rhs=wo[fc][:, half * 512:(half + 1) * 512],
                                 start=(fc == 0), stop=(fc == 7))
            nc.vector.tensor_copy(ysb[Ic][:, half * 512:(half + 1) * 512], yp[:])
        nc.sync.dma_start(out=D["out"][Ic * 128:(Ic + 1) * 128, :], in_=ysb[Ic][:])

    if dbg:
        nc.sync.dma_start(out=D["dbg_qrot0"][:], in_=qrot[0][:])
        nc.sync.dma_start(out=D["dbg_krot0"][:], in_=krot[0][:])
        nc.sync.dma_start(out=D["dbg_vsb1"][:], in_=vsb[1][:])
        nc.sync.dma_start(out=D["dbg_vsb5"][:], in_=vsb[5][:])
        nc.sync.dma_start(out=D["dbg_attg0"][:], in_=attg[0][:])
        nc.sync.dma_start(out=D["dbg_aT0"][:], in_=aT[0][:])

    es.close()


# ---------------- host side ----------------

def _make_consts():
    inv_freq = 1.0 / (10000.0 ** (np.arange(0, HD, 2, dtype=np.float64) / HD))
    pos = np.arange(S, dtype=np.float64)
    freqs = np.outer(pos, inv_freq)
    emb = np.concatenate([freqs, freqs], -1)
    return np.cos(emb).astype(np.float32), np.sin(emb).astype(np.float32)


def _rmat2():
    R = np.zeros((HD, HD), np.float32)
    for i in range(HD // 2):
        R[2 * i, 2 * i + 1] = -1.0
        R[2 * i + 1, 2 * i] = 1.0
    R2 = np.zeros((128, 128), np.float32)
    R2[0:64, 0:64] = R
    R2[64:128, 64:128] = R
    return np.ascontiguousarray(R2.T)  # lhsT so that lhsT.T @ q = R2 @ q


def build_in_maps(x, qkv_w, out_w):
    x = np.asarray(x, np.float32)
    qkv_w = np.asarray(qkv_w, np.float32)
    out_w = np.asarray(out_w, np.float32)
    cos_full, sin_full = _make_consts()

    wq = np.ascontiguousarray(qkv_w[0:EMB].T).astype(BF16)
    wk = np.ascontiguousarray(qkv_w[EMB:2 * EMB].T).astype(BF16)
    wv = np.ascontiguousarray(qkv_w[2 * EMB:3 * EMB].T).astype(BF16)
    wo = np.ascontiguousarray(out_w.T).astype(BF16)
    rmat = _rmat2().astype(BF16)
    ar = np.arange(128)
    mtri = (ar[:, None] <= ar[None, :]).astype(np.float32).astype(BF16)
    mwin = (ar[:, None] >= ar[None, :]).astype(np.float32).astype(BF16)

    in_maps = []
    for c in range(8):
        b, si = c // 4, c % 4
        xb = x[b]
        ctx = np.zeros((CTX, EMB), np.float32)
        if si > 0:
            ctx[0:128] = xb[512 * si - 128:512 * si]
        ctx[128:640] = xb[512 * si:512 * si + 512]
        ctx[640:644] = xb[0:4]
        xt = np.ascontiguousarray(ctx.T).astype(BF16)

        own_pos = np.arange(512 * si, 512 * si + 512)
        ctx_pos = np.zeros(CTX, np.int64)
        if si > 0:
            ctx_pos[0:128] = np.arange(512 * si - 128, 512 * si)
        ctx_pos[128:640] = own_pos
        ctx_pos[640:644] = np.arange(4)

        cosq = np.ascontiguousarray(np.tile(cos_full[own_pos].T, (2, 1)) * SCALE).astype(BF16)
        sinq = np.ascontiguousarray(np.tile(sin_full[own_pos].T, (2, 1)) * SCALE).astype(BF16)
        cosk = np.ascontiguousarray(np.tile(cos_full[ctx_pos].T, (2, 1))).astype(BF16)
        sink = np.ascontiguousarray(np.tile(sin_full[ctx_pos].T, (2, 1))).astype(BF16)

        m0 = mwin if si > 0 else np.zeros((128, 128), BF16)
        gmask = np.full((128, 256), 1.0 if si > 0 else 0.0, np.float32).astype(BF16)

        in_maps.append({
            "xt": xt, "wq": wq, "wk": wk, "wv": wv, "wo": wo,
            "cosq": cosq, "sinq": sinq, "cosk": cosk, "sink": sink,
            "rmat": rmat, "mtri": mtri, "mwin": mwin, "m0": m0, "gmask": gmask,
            "ident": np.eye(128, dtype=np.float32).astype(BF16),
        })
    return in_maps


_NC = None


def _get_nc():
    global _NC
    if _NC is None:
        _NC = _build_graph()
    return _NC


LAST_EXEC_NS = None
LAST_RESULTS = None


def _ensure_ntff_hook():
    """The image's antenv lacks axon_hooks; shim it so trace=True works."""
    import types
    try:
        import antenv.axon_hooks  # noqa: F401
        return
    except ImportError:
        pass
    import antenv
    mod = types.ModuleType("antenv.axon_hooks")
    state = {"hook": None}
    mod.set_axon_ntff_profile_hook = lambda h: state.__setitem__("hook", h)
    mod.get_axon_ntff_profile_hook = lambda: state["hook"]
    sys.modules["antenv.axon_hooks"] = mod
    antenv.axon_hooks = mod
    try:
        from trn_agent_boot.trn_boot import _ntff_profile_via_ctypes
        h = _ntff_profile_via_ctypes("/opt/axon/libaxon_pjrt.so")
        if h is not None:
            mod.set_axon_ntff_profile_hook(h)
    except Exception:
        pass


def _run(x, qkv_w, out_w, trace=False):
    global LAST_EXEC_NS, LAST_RESULTS
    if trace:
        _ensure_ntff_hook()
    nc = _get_nc()
    in_maps = build_in_maps(x, qkv_w, out_w)
    res = bass_utils.run_bass_kernel_spmd(nc, in_maps, core_ids=list(range(8)),
                                          trace=trace)
    LAST_EXEC_NS = res.exec_time_ns
    LAST_RESULTS = res
    y = np.zeros((B, S, EMB), np.float32)
    for c in range(8):
        b, si = c // 4, c % 4
        y[b, 512 * si:512 * si + 512] = res.results[c]["out"]
    return y


def kernel(x, qkv_w, out_w):
    return _run(x, qkv_w, out_w, trace=False)
